# revision 1
# baseline (speedup 1.0000x reference)
"""Causal multi-head attention (B=4, S=2048, D=1024, H=16) on 8 TRN2 NeuronCores.

Sharding: zero-collective. Core c handles batch b=c//2 and a causally-balanced
half of the queries (512-token quarters: half 0 = quarters {0,3}, half 1 =
{1,2} -- equal causal work). Each core computes Q for its 1024
tokens, K/V for the full sequence of its batch (duplicated across the pair of
cores -- cheaper than any collective), all 16 heads of causal attention, and
the output projection for its tokens. Host reassembles.

All 8 cores run one SPMD graph; per-core differences live only in DMA'd data.
The per-core key axis is permuted to [own tokens | other tokens] so the causal
diagonal lands at identical graph positions on every core; padding and
block-level causality enter via a per-core additive bias (per-partition,
folded into the ScalarE exp bias), and the within-diagonal triangle via a
constant 0/1 multiplicative mask.

Attention layout: transposed scores [k, q]. Per head pair and k-tile, two
N=512 score matmuls fill adjacent full PSUM banks, one fused ScalarE exp
(scale=1/8, bias=mask) reads across both banks and writes bf16 E; ctx
accumulates per pair (two heads col-packed in one bank); the softmax
denominator accumulates on DVE in bf16 with a final ones-matmul (M=64) that
both sums across partitions and broadcasts, feeding the reciprocal normalize.
Emission order pipelines attention segments against later QKV stages so the
ScalarE exp stream overlaps projection matmuls.
"""

import os
import sys

sys.path.insert(0, "/opt/trn_rl_repo")

import numpy as np
import ml_dtypes

import concourse.bass as bass
import concourse.bacc as bacc
import concourse.tile as tile
from concourse import mybir
from concourse.bass_utils import run_bass_kernel_spmd

B, S, D, H = 4, 2048, 1024, 16
HD = D // H  # 64
P = 128
NQ = S // 2  # queries per core (1024)
KC = D // P  # 8 contraction chunks
QW = 512     # query stripe width
NEG = -1e30
BF16 = mybir.dt.bfloat16
F32 = mybir.dt.float32
NPBF16 = ml_dtypes.bfloat16

# Query stripes (of width 512) owned by each half, in global order.
OWN_STRIPES = ([0, 3], [1, 2])

# k-tile positions (permuted token space) each slot's k-loop visits: the union
# over both cores of the not-entirely-masked positions. Own stripe i sits at
# ktiles {2i, 2i+1}; slot s's own tokens are ktiles {2s, 2s+1} (the diagonal).
SLOT_KTILES = [
    [0, 1, 2, 3, 8, 9, 10, 11],
    list(range(16)),
]
N_SLOTS = 2
PAD_COLS = sum(len(k) for k in SLOT_KTILES)  # 24


def _build():
    nc = bacc.Bacc()

    xt = nc.declare_dram_parameter("xt", [P, KC, S], BF16, isOutput=False)
    wq = nc.declare_dram_parameter("wq", [P, KC, D], BF16, isOutput=False)
    wk = nc.declare_dram_parameter("wk", [P, KC, D], BF16, isOutput=False)
    wv = nc.declare_dram_parameter("wv", [P, KC, D], BF16, isOutput=False)
    wo = nc.declare_dram_parameter("wo", [P, KC, D], BF16, isOutput=False)
    bqp = nc.declare_dram_parameter("bqp", [P, KC], F32, isOutput=False)
    bkp = nc.declare_dram_parameter("bkp", [P, KC], F32, isOutput=False)
    bor = nc.declare_dram_parameter("bor", [P, D], BF16, isOutput=False)
    pad = nc.declare_dram_parameter("pad", [P, PAD_COLS], F32, isOutput=False)
    tri = nc.declare_dram_parameter("tri", [P, 896], BF16, isOutput=False)
    out = nc.declare_dram_parameter("out", [NQ, D], F32, isOutput=True)

    from contextlib import ExitStack

    with tile.TileContext(nc) as tc, ExitStack() as ctx:
        wpool = ctx.enter_context(tc.tile_pool(name="wpool", bufs=1))
        xpool = ctx.enter_context(tc.tile_pool(name="xpool", bufs=2))
        bigpool = ctx.enter_context(tc.tile_pool(name="bigpool", bufs=1))
        epool = ctx.enter_context(tc.tile_pool(name="epool", bufs=5))
        dpool = ctx.enter_context(tc.tile_pool(name="dpool", bufs=2))
        spool = ctx.enter_context(tc.tile_pool(name="spool", bufs=3))
        pp_acc = ctx.enter_context(tc.tile_pool(name="pp_acc", bufs=2, space="PSUM"))
        pp_sc = ctx.enter_context(tc.tile_pool(name="pp_sc", bufs=2, space="PSUM"))
        pp_ctx = ctx.enter_context(tc.tile_pool(name="pp_ctx", bufs=2, space="PSUM"))

        # ---- constants into SBUF ----
        wq_s = wpool.tile([P, KC, D], BF16, tag="wq")
        wk_s = wpool.tile([P, KC, D], BF16, tag="wk")
        wv_s = wpool.tile([P, KC, D], BF16, tag="wv")
        wo_s = wpool.tile([P, KC, D], BF16, tag="wo")
        bq_s = wpool.tile([P, KC], F32, tag="bq")
        bk_s = wpool.tile([P, KC], F32, tag="bk")
        bo_s = wpool.tile([P, D], BF16, tag="bo")
        pad_s = wpool.tile([P, PAD_COLS], F32, tag="pad")
        tri_s = wpool.tile([P, 896], BF16, tag="tri")
        ones_s = wpool.tile([P, HD], BF16, tag="ones")
        nc.vector.memset(ones_s[:], 1.0)
        # touch Exp once at t=0 so the ~2.7us ACT table load happens inside
        # the startup DMA shadow, not at the first real softmax
        warm_s = wpool.tile([P, 1], F32, tag="warm")
        nc.vector.memset(warm_s[:], 0.0)
        nc.scalar.activation(warm_s[:], warm_s[:],
                             mybir.ActivationFunctionType.Exp, scale=1.0)

        # ---- big persistent activations ----
        qT_s = bigpool.tile([P, KC, NQ], BF16, tag="qT")     # [pairdims, pair, q]
        kT_s = bigpool.tile([P, KC, S], BF16, tag="kT")      # [pairdims, pair, k]
        v_s = bigpool.tile([P, S // P, D], BF16, tag="v")    # [k in tile, ktile, do]
        cT_s = bigpool.tile([P, KC, NQ], BF16, tag="cT")     # [pairdims, pair, q]

        def load_xt(st):
            ssl = slice(st * 512, (st + 1) * 512)
            xt_t = xpool.tile([P, KC, 512], BF16, tag="xt")
            nc.sync.dma_start(xt_t[:], xt[:, :, ssl])
            return xt_t

        def qkv_stage(st, xt_t, parts="qkv"):
            """Project permuted tokens [st*512, (st+1)*512)."""
            ssl = slice(st * 512, (st + 1) * 512)
            plan = []
            if "q" in parts and st < 2:  # own tokens -> Q first
                plan.append((wq_s, bq_s, qT_s))
            if "k" in parts:
                plan.append((wk_s, bk_s, kT_s))
            for w_s, b_s, dst in plan:
                for m in range(KC):
                    ps = pp_acc.tile([P, 512], F32, tag="acc")
                    for kc in range(KC):
                        nc.tensor.matmul(
                            ps[:], lhsT=w_s[:, kc, m * P:(m + 1) * P],
                            rhs=xt_t[:, kc, :],
                            start=(kc == 0), stop=(kc == KC - 1))
                    nc.vector.tensor_scalar_add(dst[:, m, ssl], ps[:], b_s[:, m:m + 1])
            if "v" in parts:
                for sub in range(4):
                    for dt in range(2):
                        dsl = slice(dt * 512, (dt + 1) * 512)
                        ps = pp_acc.tile([P, 512], F32, tag="acc")
                        for kc in range(KC):
                            nc.tensor.matmul(
                                ps[:], lhsT=xt_t[:, kc, sub * P:(sub + 1) * P],
                                rhs=wv_s[:, kc, dsl],
                                start=(kc == 0), stop=(kc == KC - 1))
                        nc.vector.tensor_copy(out=v_s[:, st * 4 + sub, dsl],
                                              in_=ps[:])

        pad_base = [0, 8]  # running offset of SLOT_KTILES lengths

        attn_state = {}

        def attn_pairs(slot, pairs, jlo, jhi, den_sc=False):
            """Emit k-loop segment [jlo, jhi) of the given head pairs; the
            (ctx, dacc) accumulators live in attn_state across segments."""
            qsl = slice(slot * QW, (slot + 1) * QW)
            ktiles = SLOT_KTILES[slot]
            nkt = len(ktiles)
            for pr in pairs:
                hA, hB = 2 * pr, 2 * pr + 1
                if jlo == 0:
                    ctx_new = pp_ctx.tile([P, QW], F32, tag="ctx", name="ctx_ps")
                    dacc_new = dpool.tile([P, 2 * QW], BF16, tag="dacc", name="dacc")
                    attn_state[(slot, pr)] = (ctx_new, dacc_new)
                ctx_ps, dacc = attn_state[(slot, pr)]
                for j in range(jlo, jhi):
                    m = ktiles[j]
                    ksl = slice(m * P, (m + 1) * P)
                    diag = 4 * slot <= m < 4 * slot + 4
                    # within a diagonal k-tile of shift t, queries below t*128
                    # are entirely masked -- compute only the valid sub-range
                    off = (m - 4 * slot) * P if diag else 0
                    w = QW - off
                    qsub = slice(slot * QW + off, (slot + 1) * QW)
                    # scores for both heads in adjacent full banks
                    sc = pp_sc.tile([P, 2 * QW], F32, tag="sc")
                    for q_i in range(2):
                        lo = q_i * HD
                        nc.tensor.matmul(
                            sc[:, q_i * QW + off:(q_i + 1) * QW],
                            lhsT=kT_s[lo:lo + HD, pr, ksl],
                            rhs=qT_s[lo:lo + HD, pr, qsub],
                            start=True, stop=True, tile_position=(lo, 0))
                    pcol = pad_base[slot] + j
                    e = epool.tile([P, 2 * QW], BF16, tag="e")
                    if off == 0:
                        nc.scalar.activation(e[:], sc[:],
                                             mybir.ActivationFunctionType.Exp,
                                             bias=pad_s[:, pcol:pcol + 1],
                                             scale=0.125)
                    else:
                        for q_i in range(2):
                            esl = slice(q_i * QW + off, (q_i + 1) * QW)
                            nc.scalar.activation(e[:, esl], sc[:, esl],
                                                 mybir.ActivationFunctionType.Exp,
                                                 bias=pad_s[:, pcol:pcol + 1],
                                                 scale=0.125)
                    if diag:
                        for q_i in range(2):
                            esl = slice(q_i * QW + off, (q_i + 1) * QW)
                            nc.vector.tensor_tensor(
                                e[:, esl], e[:, esl],
                                tri_s[:, 384:384 + w],
                                mybir.AluOpType.mult)
                    if j == 0:
                        nc.vector.tensor_scalar_add(dacc[:], e[:], 0.0)
                    elif off == 0:
                        nc.vector.tensor_tensor(dacc[:], dacc[:], e[:],
                                                mybir.AluOpType.add)
                    else:
                        for q_i in range(2):
                            esl = slice(q_i * QW + off, (q_i + 1) * QW)
                            nc.vector.tensor_tensor(dacc[:, esl], dacc[:, esl],
                                                    e[:, esl],
                                                    mybir.AluOpType.add)
                    st_, sp_ = (j == 0), (j == nkt - 1)
                    for q_i, h in enumerate((hA, hB)):
                        lo = q_i * HD
                        nc.tensor.matmul(
                            ctx_ps[lo:lo + HD, off:],
                            lhsT=v_s[:, m, h * HD:(h + 1) * HD],
                            rhs=e[:, q_i * QW + off:(q_i + 1) * QW],
                            start=st_, stop=sp_, tile_position=(0, lo),
                            skip_group_check=True)
                if jhi == nkt:
                    # denominator: broadcast column sums across partitions.
                    # ctx-pool den keeps the scores rotation bubble-free, but
                    # deadlocks if two unfinished pairs hold both ctx slots --
                    # split pairs use the sc pool instead.
                    if den_sc:
                        den_ps = pp_sc.tile([P, 2 * QW], F32, tag="sc",
                                            name="den_ps")
                    else:
                        den_ps = pp_ctx.tile([P, QW], F32, tag="ctx",
                                             name="den_ps")
                    for q_i in range(2):
                        lo = q_i * HD
                        nc.tensor.matmul(
                            den_ps[lo:lo + HD, 0:QW],
                            lhsT=ones_s[:],
                            rhs=dacc[:, q_i * QW:(q_i + 1) * QW],
                            start=True, stop=True, tile_position=(0, lo),
                            skip_group_check=True)
                    rden = spool.tile([P, QW], F32, tag="rden")
                    nc.vector.reciprocal(rden[:], den_ps[:, 0:QW])
                    nc.vector.tensor_tensor(cT_s[:, pr, qsl], ctx_ps[:], rden[:],
                                            mybir.AluOpType.mult)
                    del attn_state[(slot, pr)]

        def oproj(st8):
            osl = slice(st8 * P, (st8 + 1) * P)
            for dt in range(2):
                dsl = slice(dt * 512, (dt + 1) * 512)
                ps = pp_acc.tile([P, 512], F32, tag="acc")
                for kc in range(KC):
                    nc.tensor.matmul(ps[:], lhsT=cT_s[:, kc, osl],
                                     rhs=wo_s[:, kc, dsl],
                                     start=(kc == 0), stop=(kc == KC - 1))
                ob = spool.tile([P, 512], F32, tag="outsb")
                nc.vector.tensor_tensor(ob[:], ps[:], bo_s[:, dsl],
                                        mybir.AluOpType.add)
                nc.sync.dma_start(out[osl, dsl], ob[:])

        xt0 = load_xt(0)
        nc.sync.dma_start(wv_s[:, :, 0:512], wv[:, :, 0:512])
        nc.sync.dma_start(wv_s[:, :, 512:D], wv[:, :, 512:D])
        nc.sync.dma_start(wq_s[:], wq[:])
        nc.sync.dma_start(wk_s[:], wk[:])
        nc.sync.dma_start(bq_s[:], bqp[:])
        nc.sync.dma_start(bk_s[:], bkp[:])
        nc.sync.dma_start(pad_s[:], pad[:])
        nc.sync.dma_start(tri_s[:], tri[:])
        qkv_stage(0, xt0, parts="v")
        qkv_stage(0, xt0, parts="qk")
        xt2 = load_xt(2)
        nc.sync.dma_start(wo_s[:], wo[:])
        nc.sync.dma_start(bo_s[:], bor[:])
        attn_pairs(0, [0, 1], 0, 4)      # diagonal k-tiles need only stage 0
        qkv_stage(2, xt2)
        attn_pairs(0, [0], 4, 8, den_sc=True)
        attn_pairs(0, [1], 4, 8)
        attn_pairs(0, [2, 3, 4, 5, 6, 7], 0, 8)
        xt1 = load_xt(1)
        qkv_stage(1, xt1, parts="q")
        attn_pairs(1, [0, 1], 0, 4)      # k-tiles 0-3 need only stage 0
        qkv_stage(1, xt1, parts="kv")
        attn_pairs(1, [0, 1], 4, 12)     # rest of the non-stage-3 k-tiles
        xt3 = load_xt(3)
        qkv_stage(3, xt3)
        attn_pairs(1, [0], 12, 16, den_sc=True)
        attn_pairs(1, [1], 12, 16)
        attn_pairs(1, [2, 3], 0, 16)
        oproj(0)
        oproj(1)
        attn_pairs(1, [4, 5], 0, 16)
        oproj(2)
        oproj(3)
        attn_pairs(1, [6, 7], 0, 16)
        for st8 in range(4, 8):
            oproj(st8)

    nc.compile()
    return nc


def _stripe_tokens(stripes):
    return np.concatenate([np.arange(s * QW, (s + 1) * QW) for s in stripes])


def _core_inputs(c, x, padding_mask, Wq, bq, Wk, bk, Wv, bv, Wo, bo):
    b, h = c // 2, c % 2
    own_stripes = OWN_STRIPES[h]
    rest_stripes = [s for s in range(S // QW) if s not in own_stripes]
    own = _stripe_tokens(own_stripes)
    rest = _stripe_tokens(rest_stripes)
    perm = np.concatenate([own, rest])
    qlo = [s * QW for s in own_stripes]  # global start of slot s's queries

    xt = np.ascontiguousarray(
        x[b][perm].T.reshape(KC, P, S).transpose(1, 0, 2)).astype(NPBF16)

    def wl(W):
        return np.ascontiguousarray(
            W.T.reshape(KC, P, D).transpose(1, 0, 2)).astype(NPBF16)

    bqp = np.ascontiguousarray(bq.reshape(KC, P).T).astype(np.float32)
    bkp = np.ascontiguousarray(bk.reshape(KC, P).T).astype(np.float32)
    # softmax weights sum to 1, so the V bias passes through attention
    # unchanged and folds into the output bias: out += Wo @ bv + bo
    bo2 = bo + Wo @ bv
    bor = np.ascontiguousarray(np.tile(bo2[None, :], (P, 1))).astype(NPBF16)

    # pad bias [P, 24]: per (slot, loop position): 0 where key is valid
    # (unpadded and key-stripe not entirely after the queries), else -1e30.
    # The within-diagonal triangle is handled by `tri`, so diagonal tiles get
    # padding-only here.
    padb = np.zeros((P, PAD_COLS), np.float32)
    valid = padding_mask[b]  # [S] bool
    col = 0
    for slot in range(N_SLOTS):
        qhi = qlo[slot] + QW - 1
        for m in SLOT_KTILES[slot]:
            g = perm[m * P:(m + 1) * P]
            ok = valid[g] & (g <= qhi)
            padb[:, col] = np.where(ok, 0.0, NEG)
            col += 1

    # tri [P, 896]: all four diagonal shift patterns are windows of one
    # function: tri[p, u] = (p <= u - 384); shift t's mask over the valid
    # query range [t*128, 512) is the slice [384, 384 + 512 - t*128).
    kk = np.arange(P)[:, None]
    uu = np.arange(896)[None, :]
    trib = (kk <= uu - 384).astype(NPBF16)

    return {"xt": xt, "wq": wl(Wq), "wk": wl(Wk), "wv": wl(Wv), "wo": wl(Wo),
            "bqp": bqp, "bkp": bkp, "bor": bor,
            "pad": padb, "tri": np.ascontiguousarray(trib)}, own


_NC_CACHE = {}


def kernel(x, padding_mask, Wq, bq, Wk, bk, Wv, bv, Wo, bo):
    x = np.asarray(x, np.float32)
    padding_mask = np.asarray(padding_mask, bool)
    args = [np.asarray(a, np.float32) for a in (Wq, bq, Wk, bk, Wv, bv, Wo, bo)]

    if "nc" not in _NC_CACHE:
        _NC_CACHE["nc"] = _build()
    nc = _NC_CACHE["nc"]

    in_maps, owns = [], []
    for c in range(8):
        m, own = _core_inputs(c, x, padding_mask, *args)
        in_maps.append(m)
        owns.append(own)

    trace = bool(int(os.environ.get("KERNEL_TRACE", "0")))
    try:
        res = run_bass_kernel_spmd(nc, in_maps, core_ids=list(range(8)), trace=trace)
    except ModuleNotFoundError:
        # NTFF profiling hook unavailable in this environment
        res = run_bass_kernel_spmd(nc, in_maps, core_ids=list(range(8)))
    if trace and res.exec_time_ns is not None:
        print(f"HW exec time: {res.exec_time_ns} ns")
        _NC_CACHE["exec_time_ns"] = res.exec_time_ns

    full = np.empty((B, S, D), np.float32)
    for c in range(8):
        full[c // 2, owns[c]] = res.results[c]["out"]
    return full


if __name__ == "__main__":
    rng = np.random.default_rng(0)
    x = rng.standard_normal((B, S, D), dtype=np.float32)
    lengths = rng.integers(S // 2, S + 1, size=(B,))
    pm = np.arange(S)[None, :] < lengths[:, None]
    std = 0.02
    ws = {n: (rng.standard_normal((D, D), dtype=np.float32) * std)
          for n in ("Wq", "Wk", "Wv", "Wo")}
    z = np.zeros((D,), np.float32)
    out = kernel(x, pm, ws["Wq"], z, ws["Wk"], z, ws["Wv"], z, ws["Wo"], z)
    print(out.shape, out.dtype, np.abs(out).mean())



# revision 3
# speedup vs baseline: 1.3129x; 1.3129x over previous
"""Causal multi-head attention (B=4, S=2048, D=1024, H=16) on 8 TRN2 NeuronCores.

Sharding: zero-collective hybrid (batch x head-half). Core c handles batch
b=c//2 and heads [8*(c%2), 8*(c%2)+8) -- i.e. head-dim columns
hs = [512*hh, 512*hh+512) of Wq/Wk/Wv and rows hs of Wo. Each core projects
Q/K/V for ALL 2048 tokens of its batch but only its 512 head-dims, runs full
causal attention for its 8 heads, and computes a PARTIAL output projection
ctx[2048, 512] @ Wo[:, hs].T -> [2048, 1024]. The host sums each core pair's
partial outputs. Per-core matmul column-count is the ideal 1/8 share of the
whole problem (~227us of TensorE at 1 col / 2.4GHz-cycle) -- no duplicated
K/V projection work, unlike a batch x query-split sharding.

Attention layout (per head pair and 128-key k-tile, transposed scores [k, q]):
two N=512 score matmuls (one per head, PE row halves via tile_position) fill
adjacent PSUM banks; one fused ScalarE exp (scale=1/8, bias=per-key padding
mask) covers both banks and writes bf16 E; the within-diagonal triangle is a
constant 0/1 multiplicative mask; the softmax denominator accumulates on DVE
in bf16 with a final ones-matmul (M=64) that sums across partitions and
broadcasts, feeding the reciprocal normalize. Causally-dead query sub-ranges
of diagonal k-tiles are skipped (off-cut).

Schedule: a single software-pipelined stream of attention k-tiles (scores of
tile i+1 are emitted before ctx of tile i so the exp of tile i hides under
real TensorE work), with projection / output-projection PSUM fills emitted as
fillers between attention tiles at a steady rate. ScalarE's ~146us exp stream
then runs entirely under the ~227us TensorE stream.
"""

import os
import sys

sys.path.insert(0, "/opt/trn_rl_repo")

import numpy as np
import ml_dtypes

import concourse.bass as bass
import concourse.bacc as bacc
import concourse.tile as tile
from concourse import mybir
from concourse.bass_utils import run_bass_kernel_spmd

B, S, D, H = 4, 2048, 1024, 16
HD = D // H  # 64
P = 128
KC = D // P      # 8 contraction chunks for QKV projections
HH = D // 2      # 512 head-dims per core (8 heads)
KCH = HH // P    # 4 contraction chunks for the partial O-projection
NPAIR = 4        # head pairs per core
QW = 512         # query stripe width
NST = S // QW    # 4 token stripes
NKT = S // P     # 16 k-tiles
NEG = -1e30
BF16 = mybir.dt.bfloat16
F32 = mybir.dt.float32
NPBF16 = ml_dtypes.bfloat16


def _build():
    nc = bacc.Bacc()

    xt = nc.declare_dram_parameter("xt", [P, KC, S], BF16, isOutput=False)
    wq = nc.declare_dram_parameter("wq", [P, KC, HH], BF16, isOutput=False)
    wk = nc.declare_dram_parameter("wk", [P, KC, HH], BF16, isOutput=False)
    wv = nc.declare_dram_parameter("wv", [P, KC, HH], BF16, isOutput=False)
    wo = nc.declare_dram_parameter("wo", [P, KCH, D], BF16, isOutput=False)
    bqp = nc.declare_dram_parameter("bqp", [P, KCH], F32, isOutput=False)
    bkp = nc.declare_dram_parameter("bkp", [P, KCH], F32, isOutput=False)
    bor = nc.declare_dram_parameter("bor", [P, D], BF16, isOutput=False)
    pad = nc.declare_dram_parameter("pad", [P, NKT], F32, isOutput=False)
    tri = nc.declare_dram_parameter("tri", [P, 896], BF16, isOutput=False)
    out = nc.declare_dram_parameter("out", [S, D], F32, isOutput=True)

    from contextlib import ExitStack

    with tile.TileContext(nc) as tc, ExitStack() as ctx:
        wpool = ctx.enter_context(tc.tile_pool(name="wpool", bufs=1))
        bigpool = ctx.enter_context(tc.tile_pool(name="bigpool", bufs=1))
        epool = ctx.enter_context(tc.tile_pool(name="epool", bufs=5))
        dpool = ctx.enter_context(tc.tile_pool(name="dpool", bufs=2))
        spool = ctx.enter_context(tc.tile_pool(name="spool", bufs=3))
        pp_acc = ctx.enter_context(tc.tile_pool(name="pp_acc", bufs=2, space="PSUM"))
        pp_sc = ctx.enter_context(tc.tile_pool(name="pp_sc", bufs=2, space="PSUM"))
        pp_ctx = ctx.enter_context(tc.tile_pool(name="pp_ctx", bufs=2, space="PSUM"))

        # ---- constants into SBUF ----
        wq_s = wpool.tile([P, KC, HH], BF16, tag="wq")
        wk_s = wpool.tile([P, KC, HH], BF16, tag="wk")
        wv_s = wpool.tile([P, KC, HH], BF16, tag="wv")
        wo_s = wpool.tile([P, KCH, D], BF16, tag="wo")
        bq_s = wpool.tile([P, KCH], F32, tag="bq")
        bk_s = wpool.tile([P, KCH], F32, tag="bk")
        bo_s = wpool.tile([P, D], BF16, tag="bo")
        pad_s = wpool.tile([P, NKT], F32, tag="pad")
        tri_s = wpool.tile([P, 896], BF16, tag="tri")
        ones_s = wpool.tile([P, HD], BF16, tag="ones")
        nc.vector.memset(ones_s[:], 1.0)
        # touch Exp once at t=0 so the ACT table load happens inside the
        # startup DMA shadow, not at the first real softmax
        warm_s = wpool.tile([P, 1], F32, tag="warm")
        nc.vector.memset(warm_s[:], 0.0)
        nc.scalar.activation(warm_s[:], warm_s[:],
                             mybir.ActivationFunctionType.Exp, scale=1.0)

        # ---- big persistent activations ----
        xt_s = bigpool.tile([P, KC, S], BF16, tag="xt")      # [xin, chunk, t]
        qT_s = bigpool.tile([P, NPAIR, S], BF16, tag="qT")   # [pairdims, pair, q]
        kT_s = bigpool.tile([P, NPAIR, S], BF16, tag="kT")   # [pairdims, pair, k]
        v_s = bigpool.tile([P, NKT, HH], BF16, tag="v")      # [k in tile, ktile, hd]
        cT_s = bigpool.tile([P, KCH, S], BF16, tag="cT")     # [pairdims, pair, q]

        # ---- filler machinery: projection PSUM fills emitted between ----
        # ---- attention k-tiles to keep TensorE busy during exp waits  ----
        def fill_q(st, m, w_s=None, b_s=None, dst=None):
            w_s = w_s if w_s is not None else wq_s
            b_s = b_s if b_s is not None else bq_s
            dst = dst if dst is not None else qT_s
            ssl = slice(st * QW, (st + 1) * QW)
            ps = pp_acc.tile([P, QW], F32, tag="acc")
            for kc in range(KC):
                nc.tensor.matmul(
                    ps[:], lhsT=w_s[:, kc, m * P:(m + 1) * P],
                    rhs=xt_s[:, kc, ssl],
                    start=(kc == 0), stop=(kc == KC - 1),
                    skip_group_check=True)
            nc.vector.tensor_scalar_add(dst[:, m, ssl], ps[:], b_s[:, m:m + 1])

        def fill_k(st, m):
            fill_q(st, m, w_s=wk_s, b_s=bk_s, dst=kT_s)

        def fill_v(st, sub):
            ssl = slice(st * QW, (st + 1) * QW)
            ps = pp_acc.tile([P, QW], F32, tag="acc")
            for kc in range(KC):
                nc.tensor.matmul(
                    ps[:], lhsT=xt_s[:, kc, st * QW + sub * P:st * QW + (sub + 1) * P],
                    rhs=wv_s[:, kc, :],
                    start=(kc == 0), stop=(kc == KC - 1),
                    skip_group_check=True)
            nc.vector.tensor_copy(out=v_s[:, st * 4 + sub, :], in_=ps[:])

        def fill_o(tt, dt):
            osl = slice(tt * P, (tt + 1) * P)
            dsl = slice(dt * QW, (dt + 1) * QW)
            ps = pp_acc.tile([P, QW], F32, tag="acc")
            for kc in range(KCH):
                nc.tensor.matmul(ps[:], lhsT=cT_s[:, kc, osl],
                                 rhs=wo_s[:, kc, dsl],
                                 start=(kc == 0), stop=(kc == KCH - 1),
                                 skip_group_check=True)
            ob = spool.tile([P, QW], F32, tag="outsb")
            nc.vector.tensor_tensor(ob[:], ps[:], bo_s[:, dsl],
                                    mybir.AluOpType.add)
            nc.sync.dma_start(out[osl, dsl], ob[:])

        from collections import deque
        fillers = deque()
        # fillers carry a barrier key: all fillers with key <= k must be
        # drained before attention slot k starts (qkv(s) before attn(s)).
        FBIG = 99

        def pump(n):
            for _ in range(n):
                if not fillers:
                    return
                fillers.popleft()[1]()

        def drain(limit):
            while fillers and fillers[0][0] <= limit:
                fillers.popleft()[1]()

        # ---- attention tile stream, software-pipelined by one tile ----
        # Each stream item: (slot, pr, j) with j indexing k-tile m=j of
        # slot's k-loop range(4*(slot+1)).
        state = {}

        def scores_tile(slot, pr, j):
            m = j
            ksl = slice(m * P, (m + 1) * P)
            diag = 4 * slot <= m < 4 * slot + 4
            off = (m - 4 * slot) * P if diag else 0
            qsub = slice(slot * QW + off, (slot + 1) * QW)
            sc = pp_sc.tile([P, 2 * QW], F32, tag="sc")
            for q_i in range(2):
                lo = q_i * HD
                nc.tensor.matmul(
                    sc[:, q_i * QW + off:(q_i + 1) * QW],
                    lhsT=kT_s[lo:lo + HD, pr, ksl],
                    rhs=qT_s[lo:lo + HD, pr, qsub],
                    start=True, stop=True, tile_position=(lo, 0),
                    skip_group_check=True)
            e = epool.tile([P, 2 * QW], BF16, tag="e")
            if off == 0:
                nc.scalar.activation(e[:], sc[:],
                                     mybir.ActivationFunctionType.Exp,
                                     bias=pad_s[:, m:m + 1], scale=0.125)
            else:
                for q_i in range(2):
                    esl = slice(q_i * QW + off, (q_i + 1) * QW)
                    nc.scalar.activation(e[:, esl], sc[:, esl],
                                         mybir.ActivationFunctionType.Exp,
                                         bias=pad_s[:, m:m + 1], scale=0.125)
            return e, off

        def finish_tile(slot, pr, j, e, off):
            m = j
            nkt = 4 * (slot + 1)
            diag = off > 0 or (4 * slot <= m < 4 * slot + 4)
            w = QW - off
            if diag:
                for q_i in range(2):
                    esl = slice(q_i * QW + off, (q_i + 1) * QW)
                    nc.vector.tensor_tensor(
                        e[:, esl], e[:, esl], tri_s[:, 384:384 + w],
                        mybir.AluOpType.mult)
            if j == 0:
                ctx_ps = pp_ctx.tile([P, QW], F32, tag="ctx", name="ctx_ps")
                dacc = dpool.tile([P, 2 * QW], BF16, tag="dacc", name="dacc")
                state[(slot, pr)] = (ctx_ps, dacc)
            ctx_ps, dacc = state[(slot, pr)]
            if j == 0:
                nc.vector.tensor_scalar_add(dacc[:], e[:], 0.0)
            elif off == 0:
                nc.vector.tensor_tensor(dacc[:], dacc[:], e[:],
                                        mybir.AluOpType.add)
            else:
                for q_i in range(2):
                    esl = slice(q_i * QW + off, (q_i + 1) * QW)
                    nc.vector.tensor_tensor(dacc[:, esl], dacc[:, esl],
                                            e[:, esl], mybir.AluOpType.add)
            st_, sp_ = (j == 0), (j == nkt - 1)
            for q_i in range(2):
                lo = q_i * HD
                h = 2 * pr + q_i
                nc.tensor.matmul(
                    ctx_ps[lo:lo + HD, off:],
                    lhsT=v_s[:, m, h * HD:(h + 1) * HD],
                    rhs=e[:, q_i * QW + off:(q_i + 1) * QW],
                    start=st_, stop=sp_, tile_position=(0, lo),
                    skip_group_check=True)
            if j == nkt - 1:
                qsl = slice(slot * QW, (slot + 1) * QW)
                den_ps = pp_ctx.tile([P, QW], F32, tag="ctx", name="den_ps")
                for q_i in range(2):
                    lo = q_i * HD
                    nc.tensor.matmul(
                        den_ps[lo:lo + HD, 0:QW],
                        lhsT=ones_s[:],
                        rhs=dacc[:, q_i * QW:(q_i + 1) * QW],
                        start=True, stop=True, tile_position=(0, lo),
                        skip_group_check=True)
                rden = spool.tile([P, QW], F32, tag="rden")
                nc.vector.reciprocal(rden[:], den_ps[:])
                nc.vector.tensor_tensor(cT_s[:, pr, qsl], ctx_ps[:], rden[:],
                                        mybir.AluOpType.mult)
                del state[(slot, pr)]

        # ---- DMAs ----
        for st in range(NST):
            ssl = slice(st * QW, (st + 1) * QW)
            nc.sync.dma_start(xt_s[:, :, ssl], xt[:, :, ssl])
            if st == 0:
                nc.sync.dma_start(wq_s[:], wq[:])
                nc.sync.dma_start(wk_s[:], wk[:])
                nc.sync.dma_start(wv_s[:], wv[:])
                nc.sync.dma_start(bq_s[:], bqp[:])
                nc.sync.dma_start(bk_s[:], bkp[:])
                nc.sync.dma_start(pad_s[:], pad[:])
                nc.sync.dma_start(tri_s[:], tri[:])
        nc.sync.dma_start(wo_s[:], wo[:])
        nc.sync.dma_start(bo_s[:], bor[:])

        # ---- emission ----
        # qkv(0) directly (nothing to overlap with yet); interleave order
        # chosen so attention slot 0 could start asap if we wanted.
        for m in range(NPAIR):
            fill_q(0, m)
        for m in range(NPAIR):
            fill_k(0, m)
        for sub in range(4):
            fill_v(0, sub)

        # enqueue qkv stages 1..3 as fillers with barrier keys
        for st in range(1, NST):
            for m in range(NPAIR):
                fillers.append((st, (lambda st=st, m=m: fill_q(st, m))))
            for m in range(NPAIR):
                fillers.append((st, (lambda st=st, m=m: fill_k(st, m))))
            for sub in range(4):
                fillers.append((st, (lambda st=st, sub=sub: fill_v(st, sub))))

        # attention tile stream with one-tile scores lookahead
        stream = [(slot, pr, j)
                  for slot in range(NST)
                  for pr in range(NPAIR)
                  for j in range(4 * (slot + 1))]
        # fillers are pumped so that the pool drains evenly across the
        # remaining stream; oproj fills join the pool as slots complete.
        pending = None  # (slot, pr, j, e, off) awaiting its ctx emission
        emitted_o = set()

        for idx, (slot, pr, j) in enumerate(stream):
            if j == 0 and pr == 0:
                drain(slot)  # qkv(slot) must be fully emitted
            e, off = scores_tile(slot, pr, j)
            if pending is not None:
                finish_tile(*pending)
                if pending[0] != slot or pending[1] != pr:
                    sdone = (pending[0]
                             if (pending[1] == NPAIR - 1
                                 and pending[2] == 4 * (pending[0] + 1) - 1)
                             else None)
                    if sdone is not None and sdone not in emitted_o:
                        emitted_o.add(sdone)
                        for tt in range(4 * sdone, 4 * sdone + 4):
                            for dt in range(2):
                                fillers.append(
                                    (FBIG, (lambda tt=tt, dt=dt: fill_o(tt, dt))))
            pending = (slot, pr, j, e, off)
            # steady pumping: aim to drain all fillers by stream end
            remaining_tiles = len(stream) - idx
            if len(fillers) * 3 >= remaining_tiles:
                pump(1)
        finish_tile(*pending)
        drain(FBIG)
        for tt in range(12, 16):
            for dt in range(2):
                fill_o(tt, dt)

    nc.compile()
    return nc


def _core_inputs(c, x, padding_mask, Wq, bq, Wk, bk, Wv, bv, Wo, bo):
    b, hh = c // 2, c % 2
    hs = slice(HH * hh, HH * (hh + 1))

    xt = np.ascontiguousarray(
        x[b].T.reshape(KC, P, S).transpose(1, 0, 2)).astype(NPBF16)

    def wl(W, kc):  # W [kc*P rows contract, N cols out] -> [P, kc, N]
        return np.ascontiguousarray(
            W.reshape(kc, P, W.shape[1]).transpose(1, 0, 2)).astype(NPBF16)

    wql = wl(Wq[hs].T, KC)      # contract over x-dims (1024), out 512
    wkl = wl(Wk[hs].T, KC)
    wvl = wl(Wv[hs].T, KC)
    wol = wl(Wo[:, hs].T, KCH)  # contract over head-dims (512), out 1024

    bqp = np.ascontiguousarray(bq[hs].reshape(KCH, P).T).astype(np.float32)
    bkp = np.ascontiguousarray(bk[hs].reshape(KCH, P).T).astype(np.float32)
    # softmax weights sum to 1, so the V bias passes through attention
    # unchanged and folds into the output bias; bo itself is added on the
    # hh=0 core only (the host sums the pair's partial outputs).
    bo2 = Wo[:, hs] @ bv[hs] + (bo if hh == 0 else 0.0)
    bor = np.ascontiguousarray(np.tile(bo2[None, :], (P, 1))).astype(NPBF16)

    # pad bias [P, 16]: 0 where the key is unpadded, else -1e30.
    valid = padding_mask[b]  # [S] bool
    padb = np.where(valid.reshape(NKT, P).T, 0.0, NEG).astype(np.float32)

    # tri [P, 896]: tri[p, u] = (p <= u - 384); diagonal shift t uses the
    # window [384, 384 + 512 - t*128).
    kk = np.arange(P)[:, None]
    uu = np.arange(896)[None, :]
    trib = (kk <= uu - 384).astype(NPBF16)

    return {"xt": xt, "wq": wql, "wk": wkl, "wv": wvl, "wo": wol,
            "bqp": bqp, "bkp": bkp, "bor": bor,
            "pad": np.ascontiguousarray(padb), "tri": np.ascontiguousarray(trib)}


_NC_CACHE = {}


def kernel(x, padding_mask, Wq, bq, Wk, bk, Wv, bv, Wo, bo):
    x = np.asarray(x, np.float32)
    padding_mask = np.asarray(padding_mask, bool)
    args = [np.asarray(a, np.float32) for a in (Wq, bq, Wk, bk, Wv, bv, Wo, bo)]

    if "nc" not in _NC_CACHE:
        _NC_CACHE["nc"] = _build()
    nc = _NC_CACHE["nc"]

    in_maps = [_core_inputs(c, x, padding_mask, *args) for c in range(8)]

    trace = bool(int(os.environ.get("KERNEL_TRACE", "0")))
    try:
        res = run_bass_kernel_spmd(nc, in_maps, core_ids=list(range(8)), trace=trace)
    except ModuleNotFoundError:
        # NTFF profiling hook unavailable in this environment
        res = run_bass_kernel_spmd(nc, in_maps, core_ids=list(range(8)))
    if trace and res.exec_time_ns is not None:
        print(f"HW exec time: {res.exec_time_ns} ns")
        _NC_CACHE["exec_time_ns"] = res.exec_time_ns

    full = np.empty((B, S, D), np.float32)
    for b in range(B):
        full[b] = res.results[2 * b]["out"] + res.results[2 * b + 1]["out"]
    return full


if __name__ == "__main__":
    rng = np.random.default_rng(0)
    x = rng.standard_normal((B, S, D), dtype=np.float32)
    lengths = rng.integers(S // 2, S + 1, size=(B,))
    pm = np.arange(S)[None, :] < lengths[:, None]
    std = 0.02
    ws = {n: (rng.standard_normal((D, D), dtype=np.float32) * std)
          for n in ("Wq", "Wk", "Wv", "Wo")}
    z = np.zeros((D,), np.float32)
    out = kernel(x, pm, ws["Wq"], z, ws["Wk"], z, ws["Wv"], z, ws["Wo"], z)
    print(out.shape, out.dtype, np.abs(out).mean())
